# revision 1
# baseline (speedup 1.0000x reference)
"""BotGCN single-chip Trainium2 kernel (8 NeuronCores, SPMD + collectives).

Strategy (graph/data parallel, per sharding hint):
  - Nodes sharded 6250/core, padded to 6272 = 49 tiles of 128. A per-core
    node permutation balances incoming-edge counts across the 49 dst blocks.
  - Host preprocessing (indices only): permute/pad nodes, partition edges by
    (dst core, dst block, src half), compute degrees, pack gather indices
    (int16, wrapped) and dst-local one-hot keys.
  - Device per core: feature-major input projections (PE, weights stationary)
    -> per GCN layer: node-major transform X@W (X chunks stationary),
    row scale by dinv -> AllGather of Y = dinv*(XW) -> per dst block:
    dma_gather neighbor rows, one-hot (iota+is_equal) matmuls accumulate
    messages in PSUM (+ rank-1 bias seed + identity self-loop) -> dinv scale
    -> PE transpose back to feature-major -> output head.
"""

import os
import sys

if "/opt/trn_rl_repo" not in sys.path:
    sys.path.insert(0, "/opt/trn_rl_repo")

import numpy as np

import concourse.bacc as bacc
import concourse.bass as bass
import concourse.mybir as mybir
import concourse.tile as tile
from concourse import library_config
from concourse.bass_utils import run_bass_kernel_spmd
from concourse.masks import make_identity

# ---------------- problem constants ----------------
N = 50000
E = 800000
NCORE = 8
NPC = N // NCORE            # 6250 nodes per core
BLK = 49                    # dst blocks per core
NPAD = BLK * 128            # 6272 padded nodes per core
GPAD = NCORE * NPAD         # 50176 padded global nodes
HALF = GPAD // 2            # 25088 (int16-addressable gather halves)
DES = 768
F = 384                     # embedding dim
NT_W = 448                  # wide node tile (14 per core)
NW = NPAD // NT_W           # 14
CAPT = 9                    # gather tiles per (block, half-chunk)
CAP = CAPT * 128            # 1152 edge slots per (block, chunk)
IDXC = CAP // 16            # 72 idx columns per (block, chunk)

FP32 = mybir.dt.float32

# matmul compute mode: "f32" (exact, 4 cyc/row), "f32r" (fp32 storage,
# relaxed multiply, 1 cyc/row), "bf16" (bf16 storage for Y/weights/X)
MM_MODE = os.environ.get("MM_MODE", "f32r")
# debug: truncate program after phase ("PI", "T1", "AG1", "A1", "T2", "ALL")
KPHASES = os.environ.get("KPHASES", "ALL")
KBLKS = int(os.environ.get("KBLKS", str(BLK)))

_CACHED = {}


def _mm(ap):
    """Matmul operand APs are already in the storage dtype MDT."""
    return ap


if MM_MODE == "bf16":
    MDT = mybir.dt.bfloat16
elif MM_MODE == "f32r":
    MDT = mybir.dt.float32r
else:
    MDT = FP32


# ---------------- host preprocessing ----------------

def _balance_blocks(deg_lo, deg_hi):
    """Assign NPC nodes to BLK bins (cap 128 nodes, cap CAP per chunk).
    Returns pos[node] in [0, NPAD). Greedy: heaviest nodes first."""
    order = np.argsort(-(deg_lo + deg_hi), kind="stable")
    cnt = np.zeros(BLK, np.int64)
    lo = np.zeros(BLK, np.int64)
    hi = np.zeros(BLK, np.int64)
    pos = np.empty(NPC, np.int64)
    for n in order:
        dlo, dhi = deg_lo[n], deg_hi[n]
        feas = (cnt < 128) & (lo + dlo <= CAP) & (hi + dhi <= CAP)
        assert feas.any(), "block balancing failed; raise CAPT"
        score = np.where(feas, np.maximum(lo + dlo, hi + dhi) * 1000 + cnt,
                         1 << 60)
        b = int(np.argmin(score))
        pos[n] = b * 128 + cnt[b]
        cnt[b] += 1
        lo[b] += dlo
        hi[b] += dhi
    return pos


def _preprocess(edge_index):
    """All index-space preprocessing. Returns per-core packed index arrays,
    degree tensors, and the node permutation info for output unshuffling."""
    src = np.asarray(edge_index[0], np.int64)
    dst = np.asarray(edge_index[1], np.int64)
    deg = np.bincount(dst, minlength=N).astype(np.int64)

    src_core = src // NPC
    # chunk of an edge = which gather half its (padded) src lands in.
    # core c's padded ids are [c*NPAD, (c+1)*NPAD) so half = (src_core >= 4).
    e_chunk = (src_core >= NCORE // 2).astype(np.int64)

    deg_lo_all = np.bincount(dst[e_chunk == 0], minlength=N)
    deg_hi_all = np.bincount(dst[e_chunk == 1], minlength=N)

    pad_pos = np.empty(N, np.int64)  # node -> padded global position
    for c in range(NCORE):
        sl = slice(c * NPC, (c + 1) * NPC)
        pos = _balance_blocks(deg_lo_all[sl], deg_hi_all[sl])
        pad_pos[sl] = c * NPAD + pos

    sp = pad_pos[src]
    dp = pad_pos[dst]
    e_core = dp // NPAD
    e_block = (dp % NPAD) // 128
    e_dl = dp % 128
    e_idx16 = sp - e_chunk * HALF
    assert e_idx16.max() < HALF and e_idx16.min() >= 0

    # group edges by (core, block, chunk), sort by src for DMA locality
    order = np.lexsort((e_idx16, e_chunk, e_block, e_core))
    g_core = e_core[order]
    g_block = e_block[order]
    g_chunk = e_chunk[order]
    g_idx = e_idx16[order]
    g_dl = e_dl[order]

    gid = (g_core * BLK + g_block) * 2 + g_chunk  # group id 0..NCORE*BLK*2-1
    ngroups = NCORE * BLK * 2
    counts = np.bincount(gid, minlength=ngroups)
    assert counts.max() <= CAP, f"group overflow {counts.max()} > {CAP}"
    starts = np.zeros(ngroups, np.int64)
    np.cumsum(counts[:-1], out=starts[1:])
    slot_in_g = np.arange(len(gid)) - starts[gid]

    idx_slots = np.zeros((ngroups, CAP), np.int16)      # pad idx -> 0
    dl_slots = np.full((ngroups, CAP), 999.0, np.float32)  # pad dl -> no match
    idx_slots[gid, slot_in_g] = g_idx.astype(np.int16)
    dl_slots[gid, slot_in_g] = g_dl.astype(np.float32)

    per_core = []
    for c in range(NCORE):
        gs = idx_slots[c * BLK * 2:(c + 1) * BLK * 2]     # [98, CAP]
        ds = dl_slots[c * BLK * 2:(c + 1) * BLK * 2]      # [98, CAP]
        # idx16 wrapped: slot j at [j%16, j//16], tiled x8 on partitions
        w = gs.reshape(BLK * 2, IDXC, 16).transpose(2, 0, 1).reshape(
            16, BLK * 2 * IDXC)
        idx16 = np.tile(w, (8, 1)).copy()                 # [128, 98*72]
        # dst-local wrapped per tile: slot j at [j%128, j//128]
        dstl = ds.reshape(BLK * 2, CAPT, 128).transpose(2, 0, 1).reshape(
            128, BLK * 2 * CAPT).copy()                   # [128, 98*9]
        per_core.append((idx16, dstl))

    # per-core degree tensors in padded-position order
    deg1_col = np.ones((NCORE, 128, BLK), np.float32)
    deg1_row = np.ones((NCORE, 1, NPAD), np.float32)
    for c in range(NCORE):
        p = pad_pos[c * NPC:(c + 1) * NPC] - c * NPAD
        d1 = (deg[c * NPC:(c + 1) * NPC] + 1).astype(np.float32)
        deg1_col[c, p % 128, p // 128] = d1
        deg1_row[c, 0, p] = d1

    return pad_pos, per_core, deg1_col, deg1_row


# ---------------- device program ----------------

def _build():
    nc = bacc.Bacc("TRN2", target_bir_lowering=False, num_devices=NCORE)
    dt_in = MDT

    def ein(name, shape, dt=dt_in):
        return nc.dram_tensor(name, shape, dt, kind="ExternalInput")

    desT = ein("desT", [DES, NPAD])
    numT = ein("numT", [4, NPAD])
    catT = ein("catT", [3, NPAD])
    w_des = ein("w_des", [128, 6, 128])
    w_num = ein("w_num", [4, 128])
    w_cat = ein("w_cat", [3, 128])
    w_in = ein("w_in", [128, 3, F])
    w_g1 = ein("w_g1", [128, 3, F])
    w_g2 = ein("w_g2", [128, 3, F])
    w_o1 = ein("w_o1", [128, 3, F])
    w_o2 = ein("w_o2", [128, 3, 2])
    b_des = ein("b_des", [128, 1], FP32)
    b_num = ein("b_num", [128, 1], FP32)
    b_cat = ein("b_cat", [128, 1], FP32)
    b_in = ein("b_in", [128, 3], FP32)
    b_g1 = ein("b_g1", [1, F], FP32)
    b_g2 = ein("b_g2", [1, F], FP32)
    b_o1 = ein("b_o1", [128, 3], FP32)
    b_o2 = ein("b_o2", [2, 1], FP32)
    deg1c = ein("deg1c", [128, BLK], FP32)
    idx16 = ein("idx16", [128, BLK * 2 * IDXC], mybir.dt.int16)
    dstl = ein("dstl", [128, BLK * 2 * CAPT], FP32)

    out2 = nc.dram_tensor("out2", [2, NPAD], FP32, kind="ExternalOutput")

    xcat = nc.dram_tensor("xcat", [128, 3, NPAD], dt_in)
    xin = nc.dram_tensor("xin", [128, 3, NPAD], dt_in)
    yown = [nc.dram_tensor(f"y{l}own", [NPAD, F], dt_in) for l in (1, 2)]
    yall = [nc.dram_tensor(f"yall{l}", [GPAD, F], dt_in, addr_space="Shared")
            for l in (1, 2)]
    hfm = [nc.dram_tensor(f"h{l}fm", [128, 3, NPAD], dt_in) for l in (1, 2)]

    LR = mybir.ActivationFunctionType.Lrelu
    CP = mybir.ActivationFunctionType.Copy
    SQ = mybir.ActivationFunctionType.Sqrt
    EQ = mybir.AluOpType.is_equal

    with tile.TileContext(nc) as tc:
        with (
            tc.tile_pool(name="cst", bufs=1) as cst,
            tc.tile_pool(name="wide", bufs=8) as wide,
            tc.tile_pool(name="nar", bufs=6) as nar,
            tc.tile_pool(name="gp", bufs=3) as gp,
            tc.tile_pool(name="oh", bufs=3) as ohp,
            tc.tile_pool(name="pw", bufs=2, space="PSUM") as pw,
            tc.tile_pool(name="pa", bufs=2, space="PSUM") as pa,
            tc.tile_pool(name="pt", bufs=2, space="PSUM") as pt,
        ):
            nc.gpsimd.load_library(library_config.mlp)

            # ---- constants in SBUF
            iotab = cst.tile([128, CAPT, 128], FP32)
            nc.gpsimd.iota(iotab[:], pattern=[[0, CAPT], [1, 128]], base=0,
                           channel_multiplier=0,
                           allow_small_or_imprecise_dtypes=True)
            pcol = cst.tile([128, 1], FP32)
            nc.gpsimd.iota(pcol[:], pattern=[[0, 1]], base=0,
                           channel_multiplier=1,
                           allow_small_or_imprecise_dtypes=True)
            iden = cst.tile([128, 128], dt_in)
            nc.vector.tensor_scalar(out=iden[:], in0=iotab[:, 0, :],
                                    scalar1=pcol[:, 0:1], scalar2=None,
                                    op0=mybir.AluOpType.is_equal)
            idx_sb = cst.tile([128, BLK * 2 * IDXC], mybir.dt.int16)
            nc.sync.dma_start(idx_sb[:], idx16.ap())
            dstl_sb = cst.tile([128, BLK * 2 * CAPT], FP32)
            nc.sync.dma_start(dstl_sb[:], dstl.ap())

            wdes_sb = cst.tile([128, 6, 128], dt_in)
            nc.sync.dma_start(wdes_sb[:], w_des.ap())
            wnum_sb = cst.tile([4, 128], dt_in)
            nc.sync.dma_start(wnum_sb[:], w_num.ap())
            wcat_sb = cst.tile([3, 128], dt_in)
            nc.sync.dma_start(wcat_sb[:], w_cat.ap())
            win_sb = cst.tile([128, 3, F], dt_in)
            nc.sync.dma_start(win_sb[:], w_in.ap())
            wg1_sb = cst.tile([128, 3, F], dt_in)
            nc.sync.dma_start(wg1_sb[:], w_g1.ap())
            wg2_sb = cst.tile([128, 3, F], dt_in)
            nc.sync.dma_start(wg2_sb[:], w_g2.ap())
            wo1_sb = cst.tile([128, 3, F], dt_in)
            nc.sync.dma_start(wo1_sb[:], w_o1.ap())
            wo2_sb = cst.tile([128, 3, 2], dt_in)
            nc.sync.dma_start(wo2_sb[:], w_o2.ap())

            bdes_sb = cst.tile([128, 1], FP32)
            nc.sync.dma_start(bdes_sb[:], b_des.ap())
            bnum_sb = cst.tile([128, 1], FP32)
            nc.sync.dma_start(bnum_sb[:], b_num.ap())
            bcat_sb = cst.tile([128, 1], FP32)
            nc.sync.dma_start(bcat_sb[:], b_cat.ap())
            bin_sb = cst.tile([128, 3], FP32)
            nc.sync.dma_start(bin_sb[:], b_in.ap())
            bg_sb = [cst.tile([1, F], FP32, tag=f"bg{l}", name=f"bg{l}")
                     for l in (0, 1)]
            nc.sync.dma_start(bg_sb[0][:], b_g1.ap())
            nc.sync.dma_start(bg_sb[1][:], b_g2.ap())
            bo1_sb = cst.tile([128, 3], FP32)
            nc.sync.dma_start(bo1_sb[:], b_o1.ap())
            bo2_sb = cst.tile([2, 1], FP32)
            nc.sync.dma_start(bo2_sb[:], b_o2.ap())

            d1c_sb = cst.tile([128, BLK], FP32)
            nc.sync.dma_start(d1c_sb[:], deg1c.ap())
            # dinv = sqrt(1/deg1) per node (column layout)
            tmp_c = cst.tile([128, BLK], FP32)
            nc.vector.reciprocal(tmp_c[:], d1c_sb[:])
            dinv_c = cst.tile([128, BLK], FP32)
            nc.scalar.activation(dinv_c[:], tmp_c[:], SQ)
            # replicate gcn biases across partitions: brep[l] = ones x b_g
            ones_r = cst.tile([1, 128], FP32)
            nc.vector.memset(ones_r[:], 1.0)
            brep = []
            for l in (0, 1):
                psb = pt.tile([128, F], FP32, space="PSUM", tag="pbr",
                              name=f"psb{l}", bufs=1)
                nc.tensor.matmul(psb[:], lhsT=ones_r[:], rhs=bg_sb[l][:],
                                 start=True, stop=True)
                br = cst.tile([128, F], FP32, name=f"brep{l}")
                nc.vector.tensor_copy(br[:], psb[:])
                brep.append(br)

            # ---- phase P: input projections -> xcat (feature-major)
            for t in range(NW):
                ns = bass.ts(t, NT_W)
                ps_d = pw.tile([128, NT_W], FP32, space="PSUM", tag="pwide")
                for k in range(6):
                    r = wide.tile([128, NT_W], dt_in, tag="wrhs")
                    nc.sync.dma_start(r[:], desT.ap()[bass.ts(k, 128), ns])
                    nc.tensor.matmul(ps_d[:], lhsT=_mm(wdes_sb[:, k, :]),
                                     rhs=_mm(r[:]), start=(k == 0),
                                     stop=(k == 5))
                o_d = nar.tile([128, NT_W], dt_in, tag="mid")
                nc.scalar.activation(o_d[:], ps_d[:], LR, bias=bdes_sb[:, 0:1],
                                     alpha=0.01)
                nc.sync.dma_start(xcat.ap()[:, 0, ns], o_d[:])

                r_n = wide.tile([4, NT_W], dt_in, tag="wrhs")
                nc.sync.dma_start(r_n[:], numT.ap()[:, ns])
                ps_n = pw.tile([128, NT_W], FP32, space="PSUM", tag="pwide")
                nc.tensor.matmul(ps_n[:], lhsT=_mm(wnum_sb[:]), rhs=_mm(r_n[:]),
                                 start=True, stop=True)
                o_n = nar.tile([128, NT_W], dt_in, tag="mid")
                nc.scalar.activation(o_n[:], ps_n[:], LR, bias=bnum_sb[:, 0:1],
                                     alpha=0.01)
                nc.sync.dma_start(xcat.ap()[:, 1, ns], o_n[:])

                r_c = wide.tile([3, NT_W], dt_in, tag="wrhs")
                nc.sync.dma_start(r_c[:], catT.ap()[:, ns])
                ps_c = pw.tile([128, NT_W], FP32, space="PSUM", tag="pwide")
                nc.tensor.matmul(ps_c[:], lhsT=_mm(wcat_sb[:]), rhs=_mm(r_c[:]),
                                 start=True, stop=True)
                o_c = nar.tile([128, NT_W], dt_in, tag="mid")
                nc.scalar.activation(o_c[:], ps_c[:], LR, bias=bcat_sb[:, 0:1],
                                     alpha=0.01)
                nc.sync.dma_start(xcat.ap()[:, 2, ns], o_c[:])

            # ---- phase I: x = leaky(xcat @ W_in + b_in) -> xin
            for t in range(NW):
                ns = bass.ts(t, NT_W)
                rs = []
                for k in range(3):
                    r = wide.tile([128, NT_W], dt_in, tag="wrhs")
                    nc.sync.dma_start(r[:], xcat.ap()[:, k, ns])
                    rs.append(r)
                for m in range(3):
                    ps = pw.tile([128, NT_W], FP32, space="PSUM", tag="pwide")
                    for k in range(3):
                        nc.tensor.matmul(
                            ps[:], lhsT=_mm(win_sb[:, k, bass.ts(m, 128)]),
                            rhs=_mm(rs[k][:]), start=(k == 0), stop=(k == 2))
                    o = nar.tile([128, NT_W], dt_in, tag="mid")
                    nc.scalar.activation(o[:], ps[:], LR, bias=bin_sb[:, m:m + 1],
                                         alpha=0.01)
                    nc.sync.dma_start(xin.ap()[:, m, ns], o[:])

            # ---- two GCN layers
            gcn_layers = 0 if KPHASES == "PI" else (1 if KPHASES in ("T1", "AG1", "A1") else 2)
            for li in range(gcn_layers):
                src_fm = xin if li == 0 else hfm[0]
                wg = wg1_sb if li == 0 else wg2_sb
                bg = bg_sb[li]
                yo = yown[li]
                ya = yall[li]

                # transform: y = dinv * (x @ Wg)   (node-major out)
                for t in range(BLK):
                    ns = bass.ts(t, 128)
                    ps = pa.tile([128, F], FP32, space="PSUM", tag="pagg")
                    for k in range(3):
                        lx = nar.tile([128, 128], dt_in, tag="lx")
                        nc.sync.dma_start(lx[:], src_fm.ap()[:, k, ns])
                        nc.tensor.matmul(ps[:], lhsT=_mm(lx[:]),
                                         rhs=_mm(wg[:, k, :]),
                                         start=(k == 0), stop=(k == 2))
                    y_t = nar.tile([128, F], dt_in, tag="mid")
                    nc.scalar.activation(y_t[:], ps[:], CP,
                                         scale=dinv_c[:, t:t + 1])
                    nc.sync.dma_start(yo.ap()[ns, :], y_t[:])

                if KPHASES == "T1" and li == 0:
                    break
                nc.gpsimd.collective_compute(
                    "AllGather", mybir.AluOpType.bypass,
                    replica_groups=[list(range(NCORE))],
                    ins=[yo.ap()], outs=[ya.ap()])
                if KPHASES == "AG1" and li == 0:
                    break

                # aggregate per dst block
                for b in range(KBLKS):
                    ps = pa.tile([128, F], FP32, space="PSUM", tag="pagg")
                    for ch in range(2):
                        g = gp.tile([128, CAPT, F], dt_in, tag="gath")
                        src = ya.ap()[ch * HALF:(ch + 1) * HALF, :]
                        c0 = (b * 2 + ch) * IDXC
                        # dma_gather is limited to 1024 idxs (64 idx columns)
                        nc.gpsimd.dma_gather(
                            g[:, 0:8, :], src, idx_sb[:, c0:c0 + 64],
                            1024, 1024, F)
                        nc.gpsimd.dma_gather(
                            g[:, 8:CAPT, :], src, idx_sb[:, c0 + 64:c0 + IDXC],
                            CAP - 1024, CAP - 1024, F)
                        oh = ohp.tile([128, CAPT, 128], dt_in, tag="onehot")
                        dsl = dstl_sb[:, (b * 2 + ch) * CAPT:
                                      (b * 2 + ch + 1) * CAPT]
                        dsl_b = bass.AP(dsl.tensor, dsl.offset,
                                        list(dsl.ap) + [[0, 128]])
                        nc.vector.tensor_tensor(
                            out=oh[:], in0=iotab[:], in1=dsl_b, op=EQ)
                        for t in range(CAPT):
                            nc.tensor.matmul(ps[:], lhsT=_mm(oh[:, t, :]),
                                             rhs=_mm(g[:, t, :]),
                                             start=(ch == 0 and t == 0),
                                             stop=(ch == 1 and t == CAPT - 1))
                    yo_t = nar.tile([128, F], dt_in, tag="mid")
                    nc.sync.dma_start(yo_t[:], yo.ap()[bass.ts(b, 128), :])
                    s1 = nar.tile([128, F], FP32, tag="mid")
                    nc.vector.tensor_tensor(out=s1[:], in0=ps[:], in1=yo_t[:],
                                            op=mybir.AluOpType.add)
                    s2 = nar.tile([128, F], FP32, tag="mid")
                    nc.scalar.activation(s2[:], s1[:], CP,
                                         scale=dinv_c[:, b:b + 1])
                    h_t = nar.tile([128, F], dt_in, tag="mid")
                    nc.vector.tensor_tensor(out=h_t[:], in0=s2[:],
                                            in1=brep[li][:],
                                            op=mybir.AluOpType.add)
                    # transpose to feature-major
                    hf = nar.tile([128, 3, 128], dt_in, tag="hfm")
                    for k in range(3):
                        pst = pt.tile([128, 128], dt_in, space="PSUM",
                                      tag="ptr")
                        nc.tensor.transpose(pst[:], h_t[:, bass.ts(k, 128)],
                                            iden[:])
                        nc.vector.tensor_copy(hf[:, k, :], pst[:])
                    nc.sync.dma_start(hfm[li].ap()[:, :, bass.ts(b, 128)],
                                      hf[:])

            # ---- output head
            for t in range(NW if KPHASES == "ALL" else 0):
                ns = bass.ts(t, NT_W)
                rs = []
                for k in range(3):
                    r = wide.tile([128, NT_W], dt_in, tag="wrhs")
                    nc.sync.dma_start(r[:], hfm[1].ap()[:, k, ns])
                    rs.append(r)
                o1s = []
                for m in range(3):
                    ps = pw.tile([128, NT_W], FP32, space="PSUM", tag="pwide")
                    for k in range(3):
                        nc.tensor.matmul(
                            ps[:], lhsT=_mm(wo1_sb[:, k, bass.ts(m, 128)]),
                            rhs=_mm(rs[k][:]), start=(k == 0), stop=(k == 2))
                    o = nar.tile([128, NT_W], dt_in, tag="mid")
                    nc.scalar.activation(o[:], ps[:], LR,
                                         bias=bo1_sb[:, m:m + 1], alpha=0.01)
                    o1s.append(o)
                psf = pt.tile([2, NT_W], FP32, space="PSUM", tag="pfin", bufs=1)
                for k in range(3):
                    nc.tensor.matmul(psf[:], lhsT=_mm(wo2_sb[:, k, :]),
                                     rhs=_mm(o1s[k][:]),
                                     start=(k == 0), stop=(k == 2))
                of = nar.tile([2, NT_W], FP32, tag="mid")
                nc.scalar.activation(of[:], psf[:],
                                     mybir.ActivationFunctionType.Identity,
                                     bias=bo2_sb[:, 0:1])
                nc.sync.dma_start(out2.ap()[:, ns], of[:])

    nc.compile()
    return nc


# ---------------- top level ----------------

def _np(x, dt=np.float32):
    return np.ascontiguousarray(np.asarray(x), dtype=dt)


def prepare(des, tweet, num_prop, cat_prop, edge_index,
            W_des, b_des, W_num, b_num, W_cat, b_cat, W_in, b_in,
            W_g1, b_g1, W_g2, b_g2, W_o1, b_o1, W_o2, b_o2):
    """Build (or fetch cached) device program + per-core input maps."""
    try:
        import ml_dtypes
        bf16 = ml_dtypes.bfloat16
    except ImportError:
        bf16 = np.float32
    mdt = bf16 if MM_MODE == "bf16" else np.float32

    ek = tuple(np.asarray(edge_index).reshape(-1)[:16].tolist())
    if "prep" not in _CACHED or _CACHED.get("ekey") != ek:
        _CACHED["prep"] = _preprocess(edge_index)
        _CACHED["ekey"] = ek
    pad_pos, per_core, deg1_col, deg1_row = _CACHED["prep"]

    if "nc" not in _CACHED:
        _CACHED["nc"] = _build()
    nc = _CACHED["nc"]

    des = _np(des)
    num_prop = _np(num_prop)
    cat_prop = _np(cat_prop)

    # weights shared by all cores
    shared = dict(
        w_des=_np(W_des, mdt).reshape(6, 128, 128).transpose(1, 0, 2).copy(),
        w_num=_np(W_num, mdt), w_cat=_np(W_cat, mdt),
        w_in=_np(W_in, mdt).reshape(3, 128, F).transpose(1, 0, 2).copy(),
        w_g1=_np(W_g1, mdt).reshape(3, 128, F).transpose(1, 0, 2).copy(),
        w_g2=_np(W_g2, mdt).reshape(3, 128, F).transpose(1, 0, 2).copy(),
        w_o1=_np(W_o1, mdt).reshape(3, 128, F).transpose(1, 0, 2).copy(),
        w_o2=_np(W_o2, mdt).reshape(3, 128, 2).transpose(1, 0, 2).copy(),
        b_des=_np(b_des).reshape(128, 1), b_num=_np(b_num).reshape(128, 1),
        b_cat=_np(b_cat).reshape(128, 1),
        b_in=_np(b_in).reshape(3, 128).T.copy(),
        b_g1=_np(b_g1, mdt).reshape(1, F), b_g2=_np(b_g2, mdt).reshape(1, F),
        b_o1=_np(b_o1).reshape(3, 128).T.copy(),
        b_o2=_np(b_o2).reshape(2, 1),
    )

    in_maps = []
    for c in range(NCORE):
        p = pad_pos[c * NPC:(c + 1) * NPC] - c * NPAD
        dT = np.zeros((DES, NPAD), mdt)
        dT[:, p] = des[c * NPC:(c + 1) * NPC].T
        nT = np.zeros((4, NPAD), mdt)
        nT[:, p] = num_prop[c * NPC:(c + 1) * NPC].T
        cT = np.zeros((3, NPAD), mdt)
        cT[:, p] = cat_prop[c * NPC:(c + 1) * NPC].T
        idx16, dstl = per_core[c]
        in_maps.append(dict(
            desT=dT, numT=nT, catT=cT,
            deg1c=deg1_col[c],
            idx16=idx16, dstl=dstl, **shared))

    return nc, in_maps, pad_pos


def unshard(results, pad_pos):
    out = np.empty((N, 2), np.float32)
    for c in range(NCORE):
        o = results[c]["out2"]  # [2, NPAD]
        p = pad_pos[c * NPC:(c + 1) * NPC] - c * NPAD
        out[c * NPC:(c + 1) * NPC] = o[:, p].T
    return out


def kernel(**inputs):
    nc, in_maps, pad_pos = prepare(**inputs)
    res = run_bass_kernel_spmd(nc, in_maps, core_ids=list(range(NCORE)))
    return unshard(res.results, pad_pos)



# revision 4
# speedup vs baseline: 19.0866x; 19.0866x over previous
"""BotGCN single-chip Trainium2 kernel (8 NeuronCores, SPMD + collectives).

Strategy (graph/data parallel, per sharding hint):
  - Nodes sharded 6250/core, padded to 6272 = 49 tiles of 128. A per-core
    node permutation balances incoming-edge counts across the 49 dst blocks.
  - Host preprocessing (indices only): permute/pad nodes, partition edges by
    (dst core, dst block, src half), compute degrees, pack gather indices
    (int16, wrapped) and dst-local one-hot keys.
  - Device per core: feature-major input projections (PE, weights stationary)
    -> per GCN layer: node-major transform X@W (X chunks stationary),
    row scale by dinv -> AllGather of Y = dinv*(XW) -> per dst block:
    dma_gather neighbor rows, one-hot (iota+is_equal) matmuls accumulate
    messages in PSUM (+ rank-1 bias seed + identity self-loop) -> dinv scale
    -> PE transpose back to feature-major -> output head.
"""

import os
import sys

if "/opt/trn_rl_repo" not in sys.path:
    sys.path.insert(0, "/opt/trn_rl_repo")

import numpy as np

import concourse.bacc as bacc
import concourse.bass as bass
import concourse.mybir as mybir
import concourse.tile as tile
from concourse import library_config
from concourse.bass_utils import run_bass_kernel_spmd
from concourse.masks import make_identity

# ---------------- problem constants ----------------
N = 50000
E = 800000
NCORE = 8
NPC = N // NCORE            # 6250 nodes per core
BLK = 49                    # dst blocks per core
NPAD = BLK * 128            # 6272 padded nodes per core
GPAD = NCORE * NPAD         # 50176 padded global nodes
HALF = GPAD // 2            # 25088 (int16-addressable gather halves)
DES = 768
F = 384                     # embedding dim
NT_W = 448                  # wide node tile (14 per core)
NW = NPAD // NT_W           # 14
CAPT = 9                    # gather tiles per (block, half-chunk)
CAP = CAPT * 128            # 1152 edge slots per (block, chunk)
IDXC = CAP // 16            # 72 idx columns per (block, chunk)

FP32 = mybir.dt.float32

# matmul compute mode: "f32" (exact, 4 cyc/row), "f32r" (fp32 storage,
# relaxed multiply, 1 cyc/row), "bf16" (bf16 storage for Y/weights/X)
MM_MODE = os.environ.get("MM_MODE", "f32r")
# debug: truncate program after phase ("PI", "T1", "AG1", "A1", "T2", "ALL")
KPHASES = os.environ.get("KPHASES", "ALL")
KBLKS = int(os.environ.get("KBLKS", str(BLK)))
# SWDGE queue parallelism probe: dma_gather on queue (b*2+ch) % NQUEUES
NQUEUES = int(os.environ.get("NQUEUES", "1"))

_CACHED = {}


def _mm(ap):
    """Matmul operand APs are already in the storage dtype MDT."""
    return ap


if MM_MODE == "bf16":
    MDT = mybir.dt.bfloat16
elif MM_MODE == "f32r":
    MDT = mybir.dt.float32r
else:
    MDT = FP32


# ---------------- host preprocessing ----------------

def _balance_blocks(deg_lo, deg_hi):
    """Assign NPC nodes to BLK bins (cap 128 nodes, cap CAP per chunk).
    Returns pos[node] in [0, NPAD). Greedy: heaviest nodes first."""
    order = np.argsort(-(deg_lo + deg_hi), kind="stable")
    cnt = np.zeros(BLK, np.int64)
    lo = np.zeros(BLK, np.int64)
    hi = np.zeros(BLK, np.int64)
    pos = np.empty(NPC, np.int64)
    for n in order:
        dlo, dhi = deg_lo[n], deg_hi[n]
        feas = (cnt < 128) & (lo + dlo <= CAP) & (hi + dhi <= CAP)
        assert feas.any(), "block balancing failed; raise CAPT"
        score = np.where(feas, np.maximum(lo + dlo, hi + dhi) * 1000 + cnt,
                         1 << 60)
        b = int(np.argmin(score))
        pos[n] = b * 128 + cnt[b]
        cnt[b] += 1
        lo[b] += dlo
        hi[b] += dhi
    return pos


def _preprocess(edge_index):
    """All index-space preprocessing. Returns per-core packed index arrays,
    degree tensors, and the node permutation info for output unshuffling."""
    src = np.asarray(edge_index[0], np.int64)
    dst = np.asarray(edge_index[1], np.int64)
    deg = np.bincount(dst, minlength=N).astype(np.int64)

    src_core = src // NPC
    # chunk of an edge = which gather half its (padded) src lands in.
    # core c's padded ids are [c*NPAD, (c+1)*NPAD) so half = (src_core >= 4).
    e_chunk = (src_core >= NCORE // 2).astype(np.int64)

    deg_lo_all = np.bincount(dst[e_chunk == 0], minlength=N)
    deg_hi_all = np.bincount(dst[e_chunk == 1], minlength=N)

    pad_pos = np.empty(N, np.int64)  # node -> padded global position
    for c in range(NCORE):
        sl = slice(c * NPC, (c + 1) * NPC)
        pos = _balance_blocks(deg_lo_all[sl], deg_hi_all[sl])
        pad_pos[sl] = c * NPAD + pos

    sp = pad_pos[src]
    dp = pad_pos[dst]
    e_core = dp // NPAD
    e_block = (dp % NPAD) // 128
    e_dl = dp % 128
    e_idx16 = sp - e_chunk * HALF
    assert e_idx16.max() < HALF and e_idx16.min() >= 0

    # group edges by (core, block, chunk), sort by src for DMA locality
    order = np.lexsort((e_idx16, e_chunk, e_block, e_core))
    g_core = e_core[order]
    g_block = e_block[order]
    g_chunk = e_chunk[order]
    g_idx = e_idx16[order]
    g_dl = e_dl[order]

    gid = (g_core * BLK + g_block) * 2 + g_chunk  # group id 0..NCORE*BLK*2-1
    ngroups = NCORE * BLK * 2
    counts = np.bincount(gid, minlength=ngroups)
    assert counts.max() <= CAP, f"group overflow {counts.max()} > {CAP}"
    starts = np.zeros(ngroups, np.int64)
    np.cumsum(counts[:-1], out=starts[1:])
    slot_in_g = np.arange(len(gid)) - starts[gid]

    idx_slots = np.zeros((ngroups, CAP), np.int16)      # pad idx -> 0
    dl_slots = np.full((ngroups, CAP), 999.0, np.float32)  # pad dl -> no match
    idx_slots[gid, slot_in_g] = g_idx.astype(np.int16)
    dl_slots[gid, slot_in_g] = g_dl.astype(np.float32)

    per_core = []
    for c in range(NCORE):
        gs = idx_slots[c * BLK * 2:(c + 1) * BLK * 2]     # [98, CAP]
        ds = dl_slots[c * BLK * 2:(c + 1) * BLK * 2]      # [98, CAP]
        # idx16 wrapped: slot j at [j%16, j//16], tiled x8 on partitions
        w = gs.reshape(BLK * 2, IDXC, 16).transpose(2, 0, 1).reshape(
            16, BLK * 2 * IDXC)
        idx16 = np.tile(w, (8, 1)).copy()                 # [128, 98*72]
        # dst-local wrapped per tile: slot j at [j%128, j//128]
        dstl = ds.reshape(BLK * 2, CAPT, 128).transpose(2, 0, 1).reshape(
            128, BLK * 2 * CAPT).copy()                   # [128, 98*9]
        per_core.append((idx16, dstl))

    # per-core degree tensors in padded-position order
    deg1_col = np.ones((NCORE, 128, BLK), np.float32)
    deg1_row = np.ones((NCORE, 1, NPAD), np.float32)
    for c in range(NCORE):
        p = pad_pos[c * NPC:(c + 1) * NPC] - c * NPAD
        d1 = (deg[c * NPC:(c + 1) * NPC] + 1).astype(np.float32)
        deg1_col[c, p % 128, p // 128] = d1
        deg1_row[c, 0, p] = d1

    return pad_pos, per_core, deg1_col, deg1_row


# ---------------- device program ----------------

def _build():
    nc = bacc.Bacc("TRN2", target_bir_lowering=False, num_devices=NCORE,
                   num_swdge_queues=NQUEUES)
    dt_in = MDT

    def ein(name, shape, dt=dt_in):
        return nc.dram_tensor(name, shape, dt, kind="ExternalInput")

    desT = ein("desT", [DES, NPAD])
    numT = ein("numT", [4, NPAD])
    catT = ein("catT", [3, NPAD])
    w_des = ein("w_des", [128, 6, 128])
    w_num = ein("w_num", [4, 128])
    w_cat = ein("w_cat", [3, 128])
    w_in = ein("w_in", [128, 3, F])
    w_g1 = ein("w_g1", [128, 3, F])
    w_g2 = ein("w_g2", [128, 3, F])
    w_o1 = ein("w_o1", [128, 3, F])
    w_o2 = ein("w_o2", [128, 3, 2])
    b_des = ein("b_des", [128, 1], FP32)
    b_num = ein("b_num", [128, 1], FP32)
    b_cat = ein("b_cat", [128, 1], FP32)
    b_in = ein("b_in", [128, 3], FP32)
    b_g1 = ein("b_g1", [1, F], FP32)
    b_g2 = ein("b_g2", [1, F], FP32)
    b_o1 = ein("b_o1", [128, 3], FP32)
    b_o2 = ein("b_o2", [2, 1], FP32)
    deg1c = ein("deg1c", [128, BLK], FP32)
    idx16 = ein("idx16", [128, BLK * 2 * IDXC], mybir.dt.int16)
    dstl = ein("dstl", [128, BLK * 2 * CAPT], FP32)

    out2 = nc.dram_tensor("out2", [2, NPAD], FP32, kind="ExternalOutput")

    xcat = nc.dram_tensor("xcat", [128, 3, NPAD], dt_in)
    xin = nc.dram_tensor("xin", [128, 3, NPAD], dt_in)
    yown = [nc.dram_tensor(f"y{l}own", [NPAD, F], dt_in) for l in (1, 2)]
    yall = [nc.dram_tensor(f"yall{l}", [GPAD, F], dt_in, addr_space="Shared")
            for l in (1, 2)]
    hfm = [nc.dram_tensor(f"h{l}fm", [128, 3, NPAD], dt_in) for l in (1, 2)]

    LR = mybir.ActivationFunctionType.Lrelu
    CP = mybir.ActivationFunctionType.Copy
    SQ = mybir.ActivationFunctionType.Sqrt
    EQ = mybir.AluOpType.is_equal

    with tile.TileContext(nc) as tc:
        with (
            tc.tile_pool(name="cst", bufs=1) as cst,
            tc.tile_pool(name="wide", bufs=8) as wide,
            tc.tile_pool(name="nar", bufs=6) as nar,
            tc.tile_pool(name="gp", bufs=3) as gp,
            tc.tile_pool(name="oh", bufs=3) as ohp,
            tc.tile_pool(name="pw", bufs=2, space="PSUM") as pw,
            tc.tile_pool(name="pa", bufs=2, space="PSUM") as pa,
            tc.tile_pool(name="pt", bufs=2, space="PSUM") as pt,
        ):
            nc.gpsimd.load_library(library_config.mlp)

            # ---- constants in SBUF
            iotab = cst.tile([128, CAPT, 128], FP32)
            nc.gpsimd.iota(iotab[:], pattern=[[0, CAPT], [1, 128]], base=0,
                           channel_multiplier=0,
                           allow_small_or_imprecise_dtypes=True)
            pcol = cst.tile([128, 1], FP32)
            nc.gpsimd.iota(pcol[:], pattern=[[0, 1]], base=0,
                           channel_multiplier=1,
                           allow_small_or_imprecise_dtypes=True)
            iden = cst.tile([128, 128], dt_in)
            nc.vector.tensor_scalar(out=iden[:], in0=iotab[:, 0, :],
                                    scalar1=pcol[:, 0:1], scalar2=None,
                                    op0=mybir.AluOpType.is_equal)
            idx_sb = cst.tile([128, BLK * 2 * IDXC], mybir.dt.int16)
            nc.sync.dma_start(idx_sb[:], idx16.ap())
            dstl_sb = cst.tile([128, BLK * 2 * CAPT], FP32)
            nc.sync.dma_start(dstl_sb[:], dstl.ap())

            wdes_sb = cst.tile([128, 6, 128], dt_in)
            nc.sync.dma_start(wdes_sb[:], w_des.ap())
            wnum_sb = cst.tile([4, 128], dt_in)
            nc.sync.dma_start(wnum_sb[:], w_num.ap())
            wcat_sb = cst.tile([3, 128], dt_in)
            nc.sync.dma_start(wcat_sb[:], w_cat.ap())
            win_sb = cst.tile([128, 3, F], dt_in)
            nc.sync.dma_start(win_sb[:], w_in.ap())
            wg1_sb = cst.tile([128, 3, F], dt_in)
            nc.sync.dma_start(wg1_sb[:], w_g1.ap())
            wg2_sb = cst.tile([128, 3, F], dt_in)
            nc.sync.dma_start(wg2_sb[:], w_g2.ap())
            wo1_sb = cst.tile([128, 3, F], dt_in)
            nc.sync.dma_start(wo1_sb[:], w_o1.ap())
            wo2_sb = cst.tile([128, 3, 2], dt_in)
            nc.sync.dma_start(wo2_sb[:], w_o2.ap())

            bdes_sb = cst.tile([128, 1], FP32)
            nc.sync.dma_start(bdes_sb[:], b_des.ap())
            bnum_sb = cst.tile([128, 1], FP32)
            nc.sync.dma_start(bnum_sb[:], b_num.ap())
            bcat_sb = cst.tile([128, 1], FP32)
            nc.sync.dma_start(bcat_sb[:], b_cat.ap())
            bin_sb = cst.tile([128, 3], FP32)
            nc.sync.dma_start(bin_sb[:], b_in.ap())
            bg_sb = [cst.tile([1, F], FP32, tag=f"bg{l}", name=f"bg{l}")
                     for l in (0, 1)]
            nc.sync.dma_start(bg_sb[0][:], b_g1.ap())
            nc.sync.dma_start(bg_sb[1][:], b_g2.ap())
            bo1_sb = cst.tile([128, 3], FP32)
            nc.sync.dma_start(bo1_sb[:], b_o1.ap())
            bo2_sb = cst.tile([2, 1], FP32)
            nc.sync.dma_start(bo2_sb[:], b_o2.ap())

            d1c_sb = cst.tile([128, BLK], FP32)
            nc.sync.dma_start(d1c_sb[:], deg1c.ap())
            # dinv = sqrt(1/deg1) per node (column layout)
            tmp_c = cst.tile([128, BLK], FP32)
            nc.vector.reciprocal(tmp_c[:], d1c_sb[:])
            dinv_c = cst.tile([128, BLK], FP32)
            nc.scalar.activation(dinv_c[:], tmp_c[:], SQ)
            # replicate gcn biases across partitions: brep[l] = ones x b_g
            ones_r = cst.tile([1, 128], FP32)
            nc.vector.memset(ones_r[:], 1.0)
            brep = []
            for l in (0, 1):
                psb = pt.tile([128, F], FP32, space="PSUM", tag="pbr",
                              name=f"psb{l}", bufs=1)
                nc.tensor.matmul(psb[:], lhsT=ones_r[:], rhs=bg_sb[l][:],
                                 start=True, stop=True)
                br = cst.tile([128, F], FP32, name=f"brep{l}")
                nc.vector.tensor_copy(br[:], psb[:])
                brep.append(br)

            # ---- phase P: input projections -> xcat (feature-major)
            for t in range(NW):
                ns = bass.ts(t, NT_W)
                ps_d = pw.tile([128, NT_W], FP32, space="PSUM", tag="pwide")
                for k in range(6):
                    r = wide.tile([128, NT_W], dt_in, tag="wrhs")
                    nc.sync.dma_start(r[:], desT.ap()[bass.ts(k, 128), ns])
                    nc.tensor.matmul(ps_d[:], lhsT=_mm(wdes_sb[:, k, :]),
                                     rhs=_mm(r[:]), start=(k == 0),
                                     stop=(k == 5))
                o_d = nar.tile([128, NT_W], dt_in, tag="mid")
                nc.scalar.activation(o_d[:], ps_d[:], LR, bias=bdes_sb[:, 0:1],
                                     alpha=0.01)
                nc.sync.dma_start(xcat.ap()[:, 0, ns], o_d[:])

                r_n = wide.tile([4, NT_W], dt_in, tag="wrhs")
                nc.sync.dma_start(r_n[:], numT.ap()[:, ns])
                ps_n = pw.tile([128, NT_W], FP32, space="PSUM", tag="pwide")
                nc.tensor.matmul(ps_n[:], lhsT=_mm(wnum_sb[:]), rhs=_mm(r_n[:]),
                                 start=True, stop=True)
                o_n = nar.tile([128, NT_W], dt_in, tag="mid")
                nc.scalar.activation(o_n[:], ps_n[:], LR, bias=bnum_sb[:, 0:1],
                                     alpha=0.01)
                nc.sync.dma_start(xcat.ap()[:, 1, ns], o_n[:])

                r_c = wide.tile([3, NT_W], dt_in, tag="wrhs")
                nc.sync.dma_start(r_c[:], catT.ap()[:, ns])
                ps_c = pw.tile([128, NT_W], FP32, space="PSUM", tag="pwide")
                nc.tensor.matmul(ps_c[:], lhsT=_mm(wcat_sb[:]), rhs=_mm(r_c[:]),
                                 start=True, stop=True)
                o_c = nar.tile([128, NT_W], dt_in, tag="mid")
                nc.scalar.activation(o_c[:], ps_c[:], LR, bias=bcat_sb[:, 0:1],
                                     alpha=0.01)
                nc.sync.dma_start(xcat.ap()[:, 2, ns], o_c[:])

            # ---- phase I: x = leaky(xcat @ W_in + b_in) -> xin
            for t in range(NW):
                ns = bass.ts(t, NT_W)
                rs = []
                for k in range(3):
                    r = wide.tile([128, NT_W], dt_in, tag="wrhs")
                    nc.sync.dma_start(r[:], xcat.ap()[:, k, ns])
                    rs.append(r)
                for m in range(3):
                    ps = pw.tile([128, NT_W], FP32, space="PSUM", tag="pwide")
                    for k in range(3):
                        nc.tensor.matmul(
                            ps[:], lhsT=_mm(win_sb[:, k, bass.ts(m, 128)]),
                            rhs=_mm(rs[k][:]), start=(k == 0), stop=(k == 2))
                    o = nar.tile([128, NT_W], dt_in, tag="mid")
                    nc.scalar.activation(o[:], ps[:], LR, bias=bin_sb[:, m:m + 1],
                                         alpha=0.01)
                    nc.sync.dma_start(xin.ap()[:, m, ns], o[:])

            # ---- two GCN layers
            gcn_layers = 0 if KPHASES == "PI" else (1 if KPHASES in ("T1", "AG1", "A1") else 2)
            for li in range(gcn_layers):
                src_fm = xin if li == 0 else hfm[0]
                wg = wg1_sb if li == 0 else wg2_sb
                bg = bg_sb[li]
                yo = yown[li]
                ya = yall[li]

                # transform: y = dinv * (x @ Wg)   (node-major out)
                for t in range(BLK):
                    ns = bass.ts(t, 128)
                    ps = pa.tile([128, F], FP32, space="PSUM", tag="pagg")
                    for k in range(3):
                        lx = nar.tile([128, 128], dt_in, tag="lx")
                        nc.sync.dma_start(lx[:], src_fm.ap()[:, k, ns])
                        nc.tensor.matmul(ps[:], lhsT=_mm(lx[:]),
                                         rhs=_mm(wg[:, k, :]),
                                         start=(k == 0), stop=(k == 2))
                    y_t = nar.tile([128, F], dt_in, tag="mid")
                    nc.scalar.activation(y_t[:], ps[:], CP,
                                         scale=dinv_c[:, t:t + 1])
                    nc.sync.dma_start(yo.ap()[ns, :], y_t[:])

                if KPHASES == "T1" and li == 0:
                    break
                nc.gpsimd.collective_compute(
                    "AllGather", mybir.AluOpType.bypass,
                    replica_groups=[list(range(NCORE))],
                    ins=[yo.ap()], outs=[ya.ap()])
                if KPHASES == "AG1" and li == 0:
                    break

                # aggregate per dst block
                for b in range(KBLKS):
                    ps = pa.tile([128, F], FP32, space="PSUM", tag="pagg")
                    for ch in range(2):
                        g = gp.tile([128, CAPT, F], dt_in, tag="gath")
                        src = ya.ap()[ch * HALF:(ch + 1) * HALF, :]
                        c0 = (b * 2 + ch) * IDXC
                        # dma_gather is limited to 1024 idxs (64 idx columns)
                        qn = (b * 2 + ch) % NQUEUES
                        nc.gpsimd.dma_gather(
                            g[:, 0:8, :], src, idx_sb[:, c0:c0 + 64],
                            1024, 1024, F, queue_num=qn)
                        nc.gpsimd.dma_gather(
                            g[:, 8:CAPT, :], src, idx_sb[:, c0 + 64:c0 + IDXC],
                            CAP - 1024, CAP - 1024, F, queue_num=qn)
                        oh = ohp.tile([128, CAPT, 128], dt_in, tag="onehot")
                        dsl = dstl_sb[:, (b * 2 + ch) * CAPT:
                                      (b * 2 + ch + 1) * CAPT]
                        dsl_b = bass.AP(dsl.tensor, dsl.offset,
                                        list(dsl.ap) + [[0, 128]])
                        nc.vector.tensor_tensor(
                            out=oh[:], in0=iotab[:], in1=dsl_b, op=EQ)
                        for t in range(CAPT):
                            nc.tensor.matmul(ps[:], lhsT=_mm(oh[:, t, :]),
                                             rhs=_mm(g[:, t, :]),
                                             start=(ch == 0 and t == 0),
                                             stop=(ch == 1 and t == CAPT - 1))
                    yo_t = nar.tile([128, F], dt_in, tag="mid")
                    nc.sync.dma_start(yo_t[:], yo.ap()[bass.ts(b, 128), :])
                    s1 = nar.tile([128, F], FP32, tag="mid")
                    nc.vector.tensor_tensor(out=s1[:], in0=ps[:], in1=yo_t[:],
                                            op=mybir.AluOpType.add)
                    s2 = nar.tile([128, F], FP32, tag="mid")
                    nc.scalar.activation(s2[:], s1[:], CP,
                                         scale=dinv_c[:, b:b + 1])
                    h_t = nar.tile([128, F], dt_in, tag="mid")
                    nc.vector.tensor_tensor(out=h_t[:], in0=s2[:],
                                            in1=brep[li][:],
                                            op=mybir.AluOpType.add)
                    # transpose to feature-major
                    hf = nar.tile([128, 3, 128], dt_in, tag="hfm")
                    for k in range(3):
                        pst = pt.tile([128, 128], dt_in, space="PSUM",
                                      tag="ptr")
                        nc.tensor.transpose(pst[:], h_t[:, bass.ts(k, 128)],
                                            iden[:])
                        nc.vector.tensor_copy(hf[:, k, :], pst[:])
                    nc.sync.dma_start(hfm[li].ap()[:, :, bass.ts(b, 128)],
                                      hf[:])

            # ---- output head
            for t in range(NW if KPHASES == "ALL" else 0):
                ns = bass.ts(t, NT_W)
                rs = []
                for k in range(3):
                    r = wide.tile([128, NT_W], dt_in, tag="wrhs")
                    nc.sync.dma_start(r[:], hfm[1].ap()[:, k, ns])
                    rs.append(r)
                o1s = []
                for m in range(3):
                    ps = pw.tile([128, NT_W], FP32, space="PSUM", tag="pwide")
                    for k in range(3):
                        nc.tensor.matmul(
                            ps[:], lhsT=_mm(wo1_sb[:, k, bass.ts(m, 128)]),
                            rhs=_mm(rs[k][:]), start=(k == 0), stop=(k == 2))
                    o = nar.tile([128, NT_W], dt_in, tag="mid")
                    nc.scalar.activation(o[:], ps[:], LR,
                                         bias=bo1_sb[:, m:m + 1], alpha=0.01)
                    o1s.append(o)
                psf = pt.tile([2, NT_W], FP32, space="PSUM", tag="pfin", bufs=1)
                for k in range(3):
                    nc.tensor.matmul(psf[:], lhsT=_mm(wo2_sb[:, k, :]),
                                     rhs=_mm(o1s[k][:]),
                                     start=(k == 0), stop=(k == 2))
                of = nar.tile([2, NT_W], FP32, tag="mid")
                nc.scalar.activation(of[:], psf[:],
                                     mybir.ActivationFunctionType.Identity,
                                     bias=bo2_sb[:, 0:1])
                nc.sync.dma_start(out2.ap()[:, ns], of[:])

    nc.compile()
    return nc


# ---------------- top level ----------------

def _np(x, dt=np.float32):
    return np.ascontiguousarray(np.asarray(x), dtype=dt)


def prepare(des, tweet, num_prop, cat_prop, edge_index,
            W_des, b_des, W_num, b_num, W_cat, b_cat, W_in, b_in,
            W_g1, b_g1, W_g2, b_g2, W_o1, b_o1, W_o2, b_o2):
    """Build (or fetch cached) device program + per-core input maps."""
    try:
        import ml_dtypes
        bf16 = ml_dtypes.bfloat16
    except ImportError:
        bf16 = np.float32
    mdt = bf16 if MM_MODE == "bf16" else np.float32

    ek = tuple(np.asarray(edge_index).reshape(-1)[:16].tolist())
    if "prep" not in _CACHED or _CACHED.get("ekey") != ek:
        _CACHED["prep"] = _preprocess(edge_index)
        _CACHED["ekey"] = ek
    pad_pos, per_core, deg1_col, deg1_row = _CACHED["prep"]

    if "nc" not in _CACHED:
        _CACHED["nc"] = _build()
    nc = _CACHED["nc"]

    des = _np(des)
    num_prop = _np(num_prop)
    cat_prop = _np(cat_prop)

    # weights shared by all cores
    shared = dict(
        w_des=_np(W_des, mdt).reshape(6, 128, 128).transpose(1, 0, 2).copy(),
        w_num=_np(W_num, mdt), w_cat=_np(W_cat, mdt),
        w_in=_np(W_in, mdt).reshape(3, 128, F).transpose(1, 0, 2).copy(),
        w_g1=_np(W_g1, mdt).reshape(3, 128, F).transpose(1, 0, 2).copy(),
        w_g2=_np(W_g2, mdt).reshape(3, 128, F).transpose(1, 0, 2).copy(),
        w_o1=_np(W_o1, mdt).reshape(3, 128, F).transpose(1, 0, 2).copy(),
        w_o2=_np(W_o2, mdt).reshape(3, 128, 2).transpose(1, 0, 2).copy(),
        b_des=_np(b_des).reshape(128, 1), b_num=_np(b_num).reshape(128, 1),
        b_cat=_np(b_cat).reshape(128, 1),
        b_in=_np(b_in).reshape(3, 128).T.copy(),
        b_g1=_np(b_g1, mdt).reshape(1, F), b_g2=_np(b_g2, mdt).reshape(1, F),
        b_o1=_np(b_o1).reshape(3, 128).T.copy(),
        b_o2=_np(b_o2).reshape(2, 1),
    )

    in_maps = []
    for c in range(NCORE):
        p = pad_pos[c * NPC:(c + 1) * NPC] - c * NPAD
        dT = np.zeros((DES, NPAD), mdt)
        dT[:, p] = des[c * NPC:(c + 1) * NPC].T
        nT = np.zeros((4, NPAD), mdt)
        nT[:, p] = num_prop[c * NPC:(c + 1) * NPC].T
        cT = np.zeros((3, NPAD), mdt)
        cT[:, p] = cat_prop[c * NPC:(c + 1) * NPC].T
        idx16, dstl = per_core[c]
        in_maps.append(dict(
            desT=dT, numT=nT, catT=cT,
            deg1c=deg1_col[c],
            idx16=idx16, dstl=dstl, **shared))

    return nc, in_maps, pad_pos


def unshard(results, pad_pos):
    out = np.empty((N, 2), np.float32)
    for c in range(NCORE):
        o = results[c]["out2"]  # [2, NPAD]
        p = pad_pos[c * NPC:(c + 1) * NPC] - c * NPAD
        out[c * NPC:(c + 1) * NPC] = o[:, p].T
    return out


def kernel(**inputs):
    nc, in_maps, pad_pos = prepare(**inputs)
    res = run_bass_kernel_spmd(nc, in_maps, core_ids=list(range(NCORE)))
    return unshard(res.results, pad_pos)

